# revision 1
# baseline (speedup 1.0000x reference)
"""Normalized-adjacency kernel (EstimateAdj.normalize, symmetric=False) for TRN2.

out = mx * r_inv[:, None] * r_inv[None, :]   where mx = adj + I,
r_inv = rowsum(mx) ** -0.5.

Strategy (8 NeuronCores, row-sharded, raw Bass with explicit semaphores):
  - host: add 1.0 to the diagonal (O(n)), split rows into 8 shards
  - device, per core: work items are HALF-tiles [128 x n/2]
    (tile t = shard rows [t*128:(t+1)*128], halves h split the columns):
      pass 1: stream the first 11 halves through 5 SBUF slots, keep the last
              5 halves resident.  Rowsums run on the SCALAR engine
              (activation Copy with accum_out), so the DVE stays free and the
              loads, not the reduces, pace the pass.
      r_inv = 1/sqrt(rowsum) (ACT sqrt + DVE reciprocal); PE transposes
      r_inv via an identity matmul so the DRAM write of the local r_inv is
      8 contiguous 512B descriptors instead of 128 scattered 32B ones.
      AllGather local r_inv (DRAM) -> full n vector; while it is in flight
      the 5 stream slots prefetch the first 5 pass-2 halves (~10 MiB).
      pass 2: fused in-place DVE scalar_tensor_tensor per half:
              half = (half * r_inv_row_scalar) * colscale[:, h-slice]; store.
              Prefetched stream halves are processed FIRST so their stores
              complete early and un-gate the remaining reloads (the reload
              chain is bandwidth-bound, not latency-bound).
  - engines: gpsimd/Pool = loads + allgather; SP/sync = stores + small DMAs;
    DVE = fused scales; ACT = rowsums + sqrt; PE = r_inv transpose.
  - host: concatenate the 8 output shards
"""

from contextlib import ExitStack

import numpy as np

import concourse.bass as bass
import concourse.mybir as mybir
from concourse.bass_utils import run_bass_kernel_spmd

N = 8192
NCORES = 8
SHARD = N // NCORES  # 1024
P = 128
T = SHARD // P  # 8 tiles per core
H = 2  # column halves per tile

F32 = mybir.dt.float32
NSTREAM = 6  # streaming half-tile slots
NCACHE = 4  # pass-1-resident half-tile slots


def build_kernel(n=N, ncores=NCORES):
    shard = n // ncores
    tt = shard // P
    w = n // H  # half width
    items = [(t, h) for t in range(tt) for h in range(H)]  # load order
    ni = len(items)

    ncache = min(NCACHE, max(ni - NSTREAM, 0))
    nstream = min(NSTREAM, ni - ncache)
    stream_items = list(range(ni - ncache))  # indices into `items`
    cached_items = list(range(ni - ncache, ni))

    def slot_of(i):
        if i >= ni - ncache:
            return nstream + (i - (ni - ncache))
        return i % nstream

    # pass-2 order: prefetched stream halves first (their stores un-gate the
    # reloads), then cached halves, then the reloaded stream halves.
    order = (
        stream_items[:nstream] + cached_items + stream_items[nstream:]
    )

    # per-slot cumulative load-completion values (s_in[slot])
    nslots = nstream + ncache
    in_count = [0] * nslots
    in_val1 = [0] * ni
    for i in range(ni):
        in_count[slot_of(i)] += 16
        in_val1[i] = in_count[slot_of(i)]
    in_val2 = {}
    for i in stream_items:
        in_count[slot_of(i)] += 16
        in_val2[i] = in_count[slot_of(i)]

    # per-stream-slot cumulative store-completion values (s_souts[slot])
    souts_count = [0] * max(nstream, 1)
    souts_val = {}
    for i in stream_items:
        souts_count[slot_of(i)] += 16
        souts_val[i] = souts_count[slot_of(i)]

    # rowsum -> r_inv -> transpose -> DRAM chain is pipelined in two groups
    # (all-but-last tile early, last tile late) so most of it hides under the
    # tail of pass 1
    groups = [(0, tt - 1), (tt - 1, tt)] if tt >= 2 else [(0, tt)]
    ng = len(groups)

    nc = bass.Bass(num_devices=ncores)
    mx = nc.dram_tensor("mx", [shard, n], F32, kind="ExternalInput")
    eye = nc.dram_tensor("eye", [P, P], F32, kind="ExternalInput")
    out = nc.dram_tensor("out", [shard, n], F32, kind="ExternalOutput")
    cc_in = nc.dram_tensor("cc_in", [shard], F32)
    cc_out = nc.dram_tensor("cc_out", [n], F32, addr_space="Shared")

    # blocked tiling: tile t, partition p, half h -> shard row t*128 + p
    mx_v = mx.rearrange("(t p) (h w) -> t p h w", p=P, h=H)
    out_v = out.rearrange("(t p) (h w) -> t p h w", p=P, h=H)

    with ExitStack() as ctx:
        slots = [
            ctx.enter_context(nc.sbuf_tensor(f"tile{i}", [P, w], F32))
            for i in range(nslots)
        ]
        colscale = ctx.enter_context(nc.sbuf_tensor("colscale", [P, n], F32))
        eye_sb = ctx.enter_context(nc.sbuf_tensor("eye_sb", [P, P], F32))
        ps = ctx.enter_context(nc.sbuf_tensor("ps", [P, ni], F32))
        rs = ctx.enter_context(nc.sbuf_tensor("rs", [P, tt], F32))
        rinv = ctx.enter_context(nc.sbuf_tensor("rinv", [P, tt], F32))
        ptc = [
            ctx.enter_context(nc.sbuf_tensor(f"ptc{g}", [b - a, P], F32))
            for g, (a, b) in enumerate(groups)
        ]
        pt = [
            ctx.enter_context(nc.psum_tensor(f"pt{g}", [b - a, P], F32))
            for g, (a, b) in enumerate(groups)
        ]

        # per-slot loads +16; per-stream-slot stores +16; compute sems +1
        s_in = [
            ctx.enter_context(nc.semaphore(f"s_in{i}")) for i in range(nslots)
        ]
        s_souts = [
            ctx.enter_context(nc.semaphore(f"s_souts{i}"))
            for i in range(max(nstream, 1))
        ]
        s_soutc = ctx.enter_context(nc.semaphore("s_soutc"))  # cached stores
        s_eye = ctx.enter_context(nc.semaphore("s_eye"))
        s_red = ctx.enter_context(nc.semaphore("s_red"))
        s_cmb = [
            ctx.enter_context(nc.semaphore(f"s_cmb{g}")) for g in range(ng)
        ]
        s_sqrt = [
            ctx.enter_context(nc.semaphore(f"s_sqrt{g}")) for g in range(ng)
        ]
        s_rcp = ctx.enter_context(nc.semaphore("s_rcp"))
        s_tp = [
            ctx.enter_context(nc.semaphore(f"s_tp{g}")) for g in range(ng)
        ]
        s_ptc = [
            ctx.enter_context(nc.semaphore(f"s_ptc{g}")) for g in range(ng)
        ]
        s_ccin = ctx.enter_context(nc.semaphore("s_ccin"))
        s_cc = ctx.enter_context(nc.semaphore("s_cc"))
        NCS = 2 * H  # column-scale broadcast chunks (quarters)
        w2 = n // NCS
        s_cs = [
            ctx.enter_context(nc.semaphore(f"s_cs{q}")) for q in range(NCS)
        ]
        s_stt = ctx.enter_context(nc.semaphore("s_stt"))
        block = ctx.enter_context(nc.Block())

        def item_src(i):
            t, h = items[i]
            return mx_v[t, :, h]

        def item_dst(i):
            t, h = items[i]
            return out_v[t, :, h]

        @block.gpsimd
        def _(g):
            # pass 1 loads
            for i in range(ni):
                if i in in_val2 and i >= nstream:
                    g.wait_ge(s_red, i - nstream + 1)  # slot's rowsum done
                g.dma_start(slots[slot_of(i)][:, :], item_src(i)).then_inc(
                    s_in[slot_of(i)], 16
                )

            # prefetch the first pass-2 stream loads (fills the AG window)
            if stream_items:
                g.wait_ge(s_red, len(stream_items))  # stream slots all free
            for i in stream_items[:nstream]:
                g.dma_start(slots[slot_of(i)][:, :], item_src(i)).then_inc(
                    s_in[slot_of(i)], 16
                )

            g.wait_ge(s_ccin, 16 * ng)  # SP wrote local r_inv to DRAM
            g.collective_compute(
                "AllGather",
                mybir.AluOpType.bypass,
                replica_groups=[list(range(ncores))],
                ins=[cc_in[:]],
                outs=[cc_out[:]],
            ).then_inc(s_cc, 1)

            # column-scale broadcast chunks: issued here (same engine as the
            # allgather -> no cross-engine hop) and on the Pool ring so the
            # stores on the SP ring are not queued behind 4 MiB of broadcast
            g.wait_ge(s_cc, 1)
            for q in range(NCS):
                g.dma_start(
                    colscale[:, q * w2 : (q + 1) * w2],
                    cc_out[q * w2 : (q + 1) * w2].partition_broadcast(P),
                ).then_inc(s_cs[q], 16)

            # remaining pass-2 stream loads (slot free when its store landed)
            for i in stream_items[nstream:]:
                g.wait_ge(s_souts[slot_of(i)], souts_val[i] - 16)
                g.dma_start(slots[slot_of(i)][:, :], item_src(i)).then_inc(
                    s_in[slot_of(i)], 16
                )

        @block.sync
        def _(sp):
            # identity for the PE transpose
            sp.dma_start(eye_sb[:, :], eye[:, :]).then_inc(s_eye, 16)
            # local r_inv (transposed via PE, staged to SBUF) -> DRAM
            for g, (a, b) in enumerate(groups):
                sp.wait_ge(s_ptc[g], 1)
                sp.dma_start(
                    cc_in[a * P : b * P], ptc[g][:, :]
                ).then_inc(s_ccin, 16)
            # stores, in pass-2 processing order
            for k, i in enumerate(order):
                sp.wait_ge(s_stt, k + 1)
                if i in in_val2:  # streamed
                    if souts_val[i] > 16:
                        sp.wait_ge(s_souts[slot_of(i)], souts_val[i] - 16)
                    sem = s_souts[slot_of(i)]
                else:
                    sem = s_soutc
                sp.dma_start(item_dst(i), slots[slot_of(i)][:, :]).then_inc(
                    sem, 16
                )
            # all stores landed before halt
            for s_idx in range(nstream):
                sp.wait_ge(s_souts[s_idx], souts_count[s_idx])
            if ncache:
                sp.wait_ge(s_soutc, 16 * ncache)

        @block.scalar
        def _(s):
            # pass 1: rowsums via in-place Copy with free-axis accumulate.
            # Group sqrts (in place on rs) are interleaved: group g's sqrt is
            # emitted right after the copies it depends on, so early groups'
            # sqrt runs in the gaps while later copies wait on their loads.
            done = 0
            for g, (a, b) in enumerate(groups):
                for i in range(done, b * H):
                    s.wait_ge(s_in[slot_of(i)], in_val1[i])
                    s.activation(
                        slots[slot_of(i)][:, :],
                        slots[slot_of(i)][:, :],
                        mybir.ActivationFunctionType.Copy,
                        accum_out=ps[:, i : i + 1],
                    ).then_inc(s_red, 1)
                done = b * H
                if b - a == 1:
                    # single-tile group: fuse half-combine + sqrt in one ACT
                    # op (no DVE round trip): sqrt(ps_even + ps_odd)
                    # (self-wait drains this engine's accum writebacks)
                    s.wait_ge(s_red, b * H)
                    s.activation(
                        rs[:, a:b],
                        ps[:, 2 * a : 2 * a + 1],
                        mybir.ActivationFunctionType.Sqrt,
                        bias=ps[:, 2 * a + 1 : 2 * a + 2],
                        scale=1.0,
                    ).then_inc(s_sqrt[g], 1)
                else:
                    s.wait_ge(s_cmb[g], 1)
                    s.sqrt(rs[:, a:b], rs[:, a:b]).then_inc(s_sqrt[g], 1)

        @block.tensor
        def _(pe):
            # sqrt(rowsum) [128, g] -> [g, 128] in PSUM (via identity)
            pe.wait_ge(s_eye, 16)
            for g, (a, b) in enumerate(groups):
                pe.wait_ge(s_sqrt[g], 1)
                pe.transpose(
                    pt[g][:, :], rs[:, a:b], eye_sb[:, :]
                ).then_inc(s_tp[g], 1)

        @block.vector
        def _(v):
            assert H == 2
            for g, (a, b) in enumerate(groups):
                if b - a > 1:
                    # combine halves: rs[:, t] = sum_h ps[:, t*H + h]
                    # (single-tile groups are fused into the ACT sqrt)
                    v.wait_ge(s_red, b * H)
                    v.scalar_tensor_tensor(
                        rs[:, a:b],
                        ps[:, 2 * a : 2 * b : 2],
                        1.0,
                        ps[:, 2 * a + 1 : 2 * b : 2],
                        op0=mybir.AluOpType.mult,
                        op1=mybir.AluOpType.add,
                    ).then_inc(s_cmb[g], 1)
                # row-scalar r_inv for the pass-2 scales
                v.wait_ge(s_sqrt[g], 1)
                v.reciprocal(rinv[:, a:b], rs[:, a:b]).then_inc(s_rcp, 1)
                # r_inv (transposed) = 1/transpose(sqrt): one fused step out
                # of PSUM, ready for the DRAM write
                v.wait_ge(s_tp[g], 1)
                v.reciprocal(ptc[g][:, :], pt[g][:, :]).then_inc(s_ptc[g], 1)
            # pass 2: fused row+column scale, in place
            # (self-wait drains the reciprocal writebacks before stts)
            v.wait_ge(s_rcp, ng)
            cs_seen = set()
            for i in order:
                t, h = items[i]
                for q in (2 * h, 2 * h + 1):
                    if q not in cs_seen:
                        cs_seen.add(q)
                        v.wait_ge(s_cs[q], 16)
                if i in in_val2:  # streamed: wait for its pass-2 load
                    v.wait_ge(s_in[slot_of(i)], in_val2[i])
                v.scalar_tensor_tensor(
                    slots[slot_of(i)][:, :],
                    slots[slot_of(i)][:, :],
                    rinv[:, t : t + 1],
                    colscale[:, h * w : (h + 1) * w],
                    op0=mybir.AluOpType.mult,
                    op1=mybir.AluOpType.mult,
                ).then_inc(s_stt, 1)

    return nc


_NC_CACHE = {}


def _get_nc(n=N, ncores=NCORES):
    key = (n, ncores)
    if key not in _NC_CACHE:
        _NC_CACHE[key] = build_kernel(n, ncores)
    return _NC_CACHE[key]


def kernel(adj, **run_kwargs):
    adj = np.asarray(adj)
    assert adj.shape == (N, N) and adj.dtype == np.float32
    mx = adj.copy()
    idx = np.arange(N)
    mx[idx, idx] += 1.0
    eye = np.eye(P, dtype=np.float32)

    in_maps = [
        {"mx": mx[c * SHARD : (c + 1) * SHARD], "eye": eye}
        for c in range(NCORES)
    ]
    nc = _get_nc()
    try:
        res = run_bass_kernel_spmd(nc, in_maps, list(range(NCORES)), **run_kwargs)
    except Exception:
        # transient device hiccups (e.g. a wedged core from an earlier
        # process) sometimes clear on a second attempt
        import time

        time.sleep(2.0)
        res = run_bass_kernel_spmd(nc, in_maps, list(range(NCORES)), **run_kwargs)
    out = np.concatenate([res.results[c]["out"] for c in range(NCORES)], axis=0)
    if run_kwargs:
        return out, res
    return out



# revision 3
# speedup vs baseline: 1.2303x; 1.2303x over previous
"""Normalized-adjacency kernel (EstimateAdj.normalize, symmetric=False) for TRN2.

out = mx * r_inv[:, None] * r_inv[None, :]   where mx = adj + I,
r_inv = rowsum(mx) ** -0.5.

Strategy (8 NeuronCores, row-sharded, raw Bass with explicit semaphores):
  - host: add 1.0 to the diagonal (O(n)), split rows into 8 shards
  - device, per core (shard [1024, 8192], half-tiles [128 x 4096], 16 items):
      pass 1: stream items 0..14 through 3 f32 SBUF slots (loads alternate
              between the Pool and SP DMA rings so per-transfer overheads
              hide); item 15 lands in a dedicated f32 tile.  Each item is
              consumed by ONE scalar-engine activation: Copy with
              accum_out -> rowsum partial, and the Copy's `out` writes a
              bf16 replica into a persistent SBUF cache (item 15's Copy is
              in place -- it stays f32).  The whole 32 MiB shard is thus
              cached on-chip and NEVER reloaded from HBM.
      r_inv = 1/sqrt(rowsum); PE transposes sqrt(rowsum), DVE reciprocal
      writes the transposed r_inv in bf16; DMA to DRAM (cc_in, 2 KB).
      AllGather (bf16, 2 KB/core) -> full n-vector cc_out; broadcast to
      the 128-partition bf16 colscale tile in two half-width chunks.
      pass 2: per item (h-major order so colscale chunk 0 un-gates the
              first 8), DVE scalar_tensor_tensor:
              slot_f32 = (cache_bf16 * r_inv_row_scalar_f32) * colscale_bf16
              then store the f32 slot; stores alternate SP/Pool rings.
              Item 15 is scaled in place in its dedicated tile.
  - bf16 rounding of mx and colscale adds ~4e-3 relative error -- inside
    the 2e-2 gate; rowsums/r_inv row scalars stay f32.
  - host: concatenate the 8 output shards

HBM traffic per core: 32 MiB load + 32 MiB store + 2 MiB colscale (vs the
two-pass baseline's 92 MiB) -> DMA-bound floor ~190 us at 360 GB/s.
"""

from contextlib import ExitStack

import numpy as np

import concourse.bass as bass
import concourse.mybir as mybir
from concourse.bass_utils import run_bass_kernel_spmd

N = 8192
NCORES = 8
SHARD = N // NCORES  # 1024
P = 128
T = SHARD // P  # 8 tiles per core
H = 2  # column halves per tile
W = N // H  # 4096

F32 = mybir.dt.float32
BF16 = mybir.dt.bfloat16
NSLOTS = 3  # f32 streaming slots (shared by pass-1 loads and pass-2 stores)


def build_kernel():
    items = [(t, h) for t in range(T) for h in range(H)]  # load order
    ni = len(items)
    last = ni - 1  # item 15: dedicated f32 tile, processed last in pass 2
    # pass-2 order: h-major so colscale chunk 0 un-gates the first 8 items
    order = [(k % T) * H + (k // T) for k in range(ni)]
    assert order[-1] == last
    groups = [(0, T - 1), (T - 1, T)]
    ng = len(groups)

    nc = bass.Bass(num_devices=NCORES)
    mx = nc.dram_tensor("mx", [SHARD, N], F32, kind="ExternalInput")
    eye = nc.dram_tensor("eye", [P, P], F32, kind="ExternalInput")
    out = nc.dram_tensor("out", [SHARD, N], F32, kind="ExternalOutput")
    cc_in = nc.dram_tensor("cc_in", [SHARD], BF16)
    cc_out = nc.dram_tensor("cc_out", [N], BF16, addr_space="Shared")

    # blocked tiling: tile t, partition p, half h -> shard row t*128 + p
    mx_v = mx.rearrange("(t p) (h w) -> t p h w", p=P, h=H)
    out_v = out.rearrange("(t p) (h w) -> t p h w", p=P, h=H)

    with ExitStack() as ctx:
        slots = [
            ctx.enter_context(nc.sbuf_tensor(f"slot{s}", [P, W], F32))
            for s in range(NSLOTS)
        ]
        ded = ctx.enter_context(nc.sbuf_tensor("ded", [P, W], F32))
        cache = [
            ctx.enter_context(nc.sbuf_tensor(f"cache{i}", [P, W], BF16))
            for i in range(ni - 1)
        ]
        colscale = ctx.enter_context(nc.sbuf_tensor("colscale", [P, N], BF16))
        eye_sb = ctx.enter_context(nc.sbuf_tensor("eye_sb", [P, P], F32))
        ps = ctx.enter_context(nc.sbuf_tensor("ps", [P, ni], F32))
        rs = ctx.enter_context(nc.sbuf_tensor("rs", [P, T], F32))
        rinv = ctx.enter_context(nc.sbuf_tensor("rinv", [P, T], F32))
        ptc = [
            ctx.enter_context(nc.sbuf_tensor(f"ptc{g}", [b - a, P], BF16))
            for g, (a, b) in enumerate(groups)
        ]
        pt = [
            ctx.enter_context(nc.psum_tensor(f"pt{g}", [b - a, P], F32))
            for g, (a, b) in enumerate(groups)
        ]

        s_in = [
            ctx.enter_context(nc.semaphore(f"s_in{s}")) for s in range(NSLOTS)
        ]
        s_ind = ctx.enter_context(nc.semaphore("s_ind"))  # ded-tile load
        s_sout = [
            ctx.enter_context(nc.semaphore(f"s_sout{s}")) for s in range(NSLOTS)
        ]
        s_soutd = ctx.enter_context(nc.semaphore("s_soutd"))  # ded store
        s_red = ctx.enter_context(nc.semaphore("s_red"))
        s_eye = ctx.enter_context(nc.semaphore("s_eye"))
        s_cmb = [
            ctx.enter_context(nc.semaphore(f"s_cmb{g}")) for g in range(ng)
        ]
        s_sqrt = [
            ctx.enter_context(nc.semaphore(f"s_sqrt{g}")) for g in range(ng)
        ]
        s_rcp = ctx.enter_context(nc.semaphore("s_rcp"))
        s_tp = [ctx.enter_context(nc.semaphore(f"s_tp{g}")) for g in range(ng)]
        s_ptc = [
            ctx.enter_context(nc.semaphore(f"s_ptc{g}")) for g in range(ng)
        ]
        s_ccin = ctx.enter_context(nc.semaphore("s_ccin"))
        s_cc = ctx.enter_context(nc.semaphore("s_cc"))
        s_cs = [ctx.enter_context(nc.semaphore(f"s_cs{q}")) for q in range(H)]
        s_stt = ctx.enter_context(nc.semaphore("s_stt"))
        block = ctx.enter_context(nc.Block())

        def load_src(i):
            t, h = items[i]
            return mx_v[t, :, h]

        def store_dst(i):
            t, h = items[i]
            return out_v[t, :, h]

        def in_tile(i):
            return ded if i == last else slots[i % NSLOTS]

        def in_sem_val(i):
            if i == last:
                return s_ind, 16
            return s_in[i % NSLOTS], 16 * (i // NSLOTS + 1)

        def out_tile(k):
            return ded if k == ni - 1 else slots[k % NSLOTS]

        # loads: even items on Pool, odd on SP.  Slot reuse is safe across
        # rings: load of item i (into slot i%3) waits until the scalar
        # engine consumed item i-3, which transitively orders the s_in
        # increments of a slot.
        @block.gpsimd
        def _(g):
            for i in range(0, ni, 2):
                if i >= NSLOTS and i != last:
                    g.wait_ge(s_red, i - (NSLOTS - 1))  # prev occupant read
                sem, _ = in_sem_val(i)
                g.dma_start(in_tile(i)[:, :], load_src(i)).then_inc(sem, 16)
            # local r_inv (transposed, bf16) -> DRAM, group 0
            g.wait_ge(s_ptc[0], 1)
            g.dma_start(cc_in[0 : (T - 1) * P], ptc[0][:, :]).then_inc(
                s_ccin, 16
            )
            g.wait_ge(s_ccin, 16 * ng)
            g.collective_compute(
                "AllGather",
                mybir.AluOpType.bypass,
                replica_groups=[list(range(NCORES))],
                ins=[cc_in[:]],
                outs=[cc_out[:]],
            ).then_inc(s_cc, 1)
            g.wait_ge(s_cc, 1)
            for q in range(H):
                g.dma_start(
                    colscale[:, q * W : (q + 1) * W],
                    cc_out[q * W : (q + 1) * W].partition_broadcast(P),
                ).then_inc(s_cs[q], 16)
            for k in range(1, ni, 2):
                g.wait_ge(s_stt, k + 1)
                sem = s_soutd if k == ni - 1 else s_sout[k % NSLOTS]
                g.dma_start(store_dst(order[k]), out_tile(k)[:, :]).then_inc(
                    sem, 16
                )

        @block.sync
        def _(sp):
            sp.dma_start(eye_sb[:, :], eye[:, :]).then_inc(s_eye, 16)
            for i in range(1, ni, 2):
                if i >= NSLOTS and i != last:
                    sp.wait_ge(s_red, i - (NSLOTS - 1))
                sem, _ = in_sem_val(i)
                sp.dma_start(in_tile(i)[:, :], load_src(i)).then_inc(sem, 16)
            # local r_inv group 1 (last tile) -> DRAM
            sp.wait_ge(s_ptc[1], 1)
            sp.dma_start(cc_in[(T - 1) * P : T * P], ptc[1][:, :]).then_inc(
                s_ccin, 16
            )
            for k in range(0, ni, 2):
                sp.wait_ge(s_stt, k + 1)
                sp.dma_start(store_dst(order[k]), out_tile(k)[:, :]).then_inc(
                    s_sout[k % NSLOTS], 16
                )
            # all stores landed before halt
            for s in range(NSLOTS):
                sp.wait_ge(s_sout[s], 16 * 5)
            sp.wait_ge(s_soutd, 16)

        @block.scalar
        def _(s):
            # pass 1: rowsum partials via Copy-with-accum; the Copy output
            # IS the bf16 cache write (no extra op for the on-chip replica);
            # item 15 copies in place and stays f32 in its dedicated tile
            done = 0
            for gi, (a, b) in enumerate(groups):
                for i in range(done, b * H):
                    sem, val = in_sem_val(i)
                    s.wait_ge(sem, val)
                    dst = in_tile(i) if i == last else cache[i]
                    s.activation(
                        dst[:, :],
                        in_tile(i)[:, :],
                        mybir.ActivationFunctionType.Copy,
                        accum_out=ps[:, i : i + 1],
                    ).then_inc(s_red, 1)
                done = b * H
                if b - a == 1:
                    # single-tile group: fused halves-combine + sqrt
                    # (self-wait drains this engine's accum writebacks)
                    s.wait_ge(s_red, b * H)
                    s.activation(
                        rs[:, a:b],
                        ps[:, 2 * a : 2 * a + 1],
                        mybir.ActivationFunctionType.Sqrt,
                        bias=ps[:, 2 * a + 1 : 2 * a + 2],
                        scale=1.0,
                    ).then_inc(s_sqrt[gi], 1)
                else:
                    s.wait_ge(s_cmb[gi], 1)
                    s.sqrt(rs[:, a:b], rs[:, a:b]).then_inc(s_sqrt[gi], 1)

        @block.tensor
        def _(pe):
            # sqrt(rowsum) [128, g] -> [g, 128] in PSUM (via identity)
            pe.wait_ge(s_eye, 16)
            for gi, (a, b) in enumerate(groups):
                pe.wait_ge(s_sqrt[gi], 1)
                pe.transpose(pt[gi][:, :], rs[:, a:b], eye_sb[:, :]).then_inc(
                    s_tp[gi], 1
                )

        @block.vector
        def _(v):
            assert H == 2
            for gi, (a, b) in enumerate(groups):
                if b - a > 1:
                    # combine halves: rs[:, t] = sum_h ps[:, t*H + h]
                    v.wait_ge(s_red, b * H)
                    v.scalar_tensor_tensor(
                        rs[:, a:b],
                        ps[:, 2 * a : 2 * b : 2],
                        1.0,
                        ps[:, 2 * a + 1 : 2 * b : 2],
                        op0=mybir.AluOpType.mult,
                        op1=mybir.AluOpType.add,
                    ).then_inc(s_cmb[gi], 1)
                # row-scalar r_inv (f32) for the pass-2 scales
                v.wait_ge(s_sqrt[gi], 1)
                v.reciprocal(rinv[:, a:b], rs[:, a:b]).then_inc(s_rcp, 1)
                # transposed r_inv in bf16, ready for the cc_in DRAM write
                v.wait_ge(s_tp[gi], 1)
                with nc.allow_low_precision("bf16 column scale is in-gate"):
                    v.reciprocal(ptc[gi][:, :], pt[gi][:, :]).then_inc(
                        s_ptc[gi], 1
                    )
            # pass 2: fused row+column scale from the bf16 cache into the
            # f32 slot, which the store rings drain (item 15: in place)
            v.wait_ge(s_rcp, ng)
            for k in range(ni):
                i = order[k]
                t, h = items[i]
                if k % T == 0:
                    v.wait_ge(s_cs[k // T], 16)
                if k >= NSLOTS and k != ni - 1:
                    v.wait_ge(s_sout[k % NSLOTS], 16 * (k // NSLOTS))
                src = in_tile(i) if i == last else cache[i]
                v.scalar_tensor_tensor(
                    out_tile(k)[:, :],
                    src[:, :],
                    rinv[:, t : t + 1],
                    colscale[:, h * W : (h + 1) * W],
                    op0=mybir.AluOpType.mult,
                    op1=mybir.AluOpType.mult,
                ).then_inc(s_stt, 1)

    return nc


_NC_CACHE = {}


def _get_nc():
    if "nc" not in _NC_CACHE:
        _NC_CACHE["nc"] = build_kernel()
    return _NC_CACHE["nc"]


def kernel(adj, **run_kwargs):
    adj = np.asarray(adj)
    assert adj.shape == (N, N) and adj.dtype == np.float32
    mx = adj.copy()
    idx = np.arange(N)
    mx[idx, idx] += 1.0
    eye = np.eye(P, dtype=np.float32)

    in_maps = [
        {"mx": mx[c * SHARD : (c + 1) * SHARD], "eye": eye}
        for c in range(NCORES)
    ]
    nc = _get_nc()
    try:
        res = run_bass_kernel_spmd(nc, in_maps, list(range(NCORES)), **run_kwargs)
    except Exception:
        # transient device hiccups (e.g. a wedged core from an earlier
        # process) sometimes clear on a second attempt
        import time

        time.sleep(2.0)
        res = run_bass_kernel_spmd(nc, in_maps, list(range(NCORES)), **run_kwargs)
    out = np.concatenate([res.results[c]["out"] for c in range(NCORES)], axis=0)
    if run_kwargs:
        return out, res
    return out
